# revision 1
# baseline (speedup 1.0000x reference)
"""2-layer LSTM encoder (batch collapsed into recurrence) on TRN2.

Single-core full-width implementation (the axon/PJRT stack on this pod
cannot execute remote-DMA descriptor instructions, and ncfw collectives
cost ~270us per call here — so no viable per-step cross-core exchange).

  GEMM0: x_pre0 = seq @ W_ih0.T + b0    (on-device, fp32 -> bf16 to HBM)
  rec0 : h0(t) = cell(x_pre0(t) + W_hh0 @ h0(t-1))   1008 steps, For_i
  GEMM1: x_pre1 = H0 @ W_ih1.T + b1     (bf16)
  rec1 : h1(t) = cell(x_pre1(t) + W_hh1 @ h1(t-1))

Layout per step: 6144 gate rows as [128 partitions x 48 psum cols]
(cols 0..35 = (i,f,o) x 12 unit-blocks; 36..47 = g x 12). The cell runs
across partitions; h [128,12] bf16 feeds the next matvec directly.
Weights stationary (bf16, fast-weight-load), fp32 PSUM accumulation.
The big weight slot (144KB/partition) is re-loaded per phase
(W_hh0 -> W_ih1 -> W_hh1); x_pre streams through HBM in chunks.
"""

import sys

sys.path.insert(0, "/opt/trn_rl_repo")
import numpy as np
import ml_dtypes
import concourse.bass as bass
import concourse.bacc as bacc
import concourse.mybir as mybir
from concourse import tile
from concourse.tile_rust import add_dep_helper
from contextlib import ExitStack

F32 = mybir.dt.float32
BF16 = mybir.dt.bfloat16
AF = mybir.ActivationFunctionType

B, T, D, H = 16, 64, 256, 1536
NB = H // 128          # 12 unit blocks
NJ = 4 * NB            # 48 psum cols
NK = H // 128          # 12 K-tiles (H contraction)
KD = D // 128          # 2 K-tiles (D contraction)

_IOFF, _FOFF, _GOFF, _OOFF = 0, H, 2 * H, 3 * H


def col_gate(j):
    if j < 36:
        return [_IOFF, _FOFF, _OOFF][j % 3], j // 3
    return _GOFF, j - 36


def gate_rows(j):
    goff, blk = col_gate(j)
    return np.arange(goff + 128 * blk, goff + 128 * blk + 128)


def pack_lhsT(W, nk):
    out = np.zeros((128, nk * NJ * 128), dtype=W.dtype)
    for k in range(nk):
        for j in range(NJ):
            out[:, (k * NJ + j) * 128 : (k * NJ + j + 1) * 128] = W[
                gate_rows(j), 128 * k : 128 * (k + 1)
            ].T
    return out


def pack_bias(b):
    out = np.zeros((1, NJ * 128), dtype=b.dtype)
    for j in range(NJ):
        out[0, 128 * j : 128 * (j + 1)] = b[gate_rows(j)]
    return out


def prep_inputs(batch, W_ih0, W_hh0, b_ih0, b_hh0, W_ih1, W_hh1, b_ih1, b_hh1,
                S=None):
    bf = ml_dtypes.bfloat16
    seq = np.ascontiguousarray(
        batch[:, 1:, :].transpose(1, 0, 2).reshape(-1, D)
    ).astype(np.float32)
    if S is not None:
        seq = seq[:S]
    S = seq.shape[0]
    seqt = np.ascontiguousarray(seq.T)
    b0 = (b_ih0 + b_hh0).astype(np.float32)
    b1 = (b_ih1 + b_hh1).astype(np.float32)
    m = {
        "seqt": np.ascontiguousarray(
            seqt.reshape(KD, 128, S).transpose(1, 0, 2).reshape(128, KD * S)
        ).astype(bf),
        "wih0t": pack_lhsT(W_ih0.astype(bf), KD),
        "wih0b": pack_bias(b0.astype(bf)),
        "whh0t": pack_lhsT(W_hh0.astype(bf), NK),
        "wih1t": pack_lhsT(W_ih1.astype(bf), NK),
        "wih1b": pack_bias(b1.astype(bf)),
        "whh1t": pack_lhsT(W_hh1.astype(bf), NK),
    }
    return m, S


def build(S=1008, CHK=42, unroll=2):
    assert S % CHK == 0 and CHK % unroll == 0
    NCHU = S // CHK
    GCH = 504 if S % 504 == 0 else S  # GEMM chunk (psum bank)
    NGC = S // GCH

    nc = bacc.Bacc(
        "TRN2",
        target_bir_lowering=False,
        debug=False,
        detect_race_conditions=False,
        num_devices=1,
    )

    seqt_e = nc.declare_dram_parameter("seqt", [128, KD * S], BF16, isOutput=False)
    wih0t_e = nc.declare_dram_parameter("wih0t", [128, KD * NJ * 128], BF16, isOutput=False)
    wih0b_e = nc.declare_dram_parameter("wih0b", [1, NJ * 128], BF16, isOutput=False)
    whh0t_e = nc.declare_dram_parameter("whh0t", [128, NK * NJ * 128], BF16, isOutput=False)
    wih1t_e = nc.declare_dram_parameter("wih1t", [128, NK * NJ * 128], BF16, isOutput=False)
    wih1b_e = nc.declare_dram_parameter("wih1b", [1, NJ * 128], BF16, isOutput=False)
    whh1t_e = nc.declare_dram_parameter("whh1t", [128, NK * NJ * 128], BF16, isOutput=False)
    hc_e = nc.declare_dram_parameter("hc", [128, 4 * NB], F32, isOutput=True)

    # x_pre staging in HBM, j-major: [128, NJ * S] with col j*S + t (bf16)
    xp0_d = nc.dram_tensor("xp0d", [128, NJ * S], BF16)
    xp1_d = nc.dram_tensor("xp1d", [128, NJ * S], BF16)

    with tile.TileContext(nc) as tc, ExitStack() as ctx:
        pool = ctx.enter_context(tc.tile_pool(name="main", bufs=1))
        pp = ctx.enter_context(tc.tile_pool(name="ps", bufs=2, space="PSUM"))

        gpool_cm = tc.tile_pool(name="g0", bufs=1)  # freed after GEMM0
        gpool = gpool_cm.__enter__()
        wih0t = gpool.tile([128, KD * NJ * 128], BF16, tag="wih0t")
        wih0b = pool.tile([1, NJ * 128], BF16, tag="wih0b")
        wih1b = pool.tile([1, NJ * 128], BF16, tag="wih1b")
        seqt = gpool.tile([128, KD * S], BF16, tag="seqt")
        arch = pool.tile([128, NB * S], BF16, tag="arch")     # H0 archive
        xpc = [pool.tile([128, NJ * CHK], BF16, tag=f"xpc{i}", name=f"xpc{i}")
               for i in range(2)]
        gstg = pool.tile([128, GCH], BF16, tag="gstg")
        ones16 = pool.tile([1, S], BF16, tag="ones16")
        cst = pool.tile([128, NB], F32, tag="cst")
        hfin = pool.tile([128, 4 * NB], F32, tag="hfin")
        hbf = pool.tile([128, NB], BF16, tag="hbf")
        gates = pool.tile([128, NJ], F32, tag="gates")
        sig = pool.tile([128, 36], F32, tag="sig")
        gt = pool.tile([128, NB], F32, tag="gt")
        th = pool.tile([128, NB], F32, tag="th")
        t1 = pool.tile([128, NB], F32, tag="t1")
        t2 = pool.tile([128, NB], F32, tag="t2")
        hf = pool.tile([128, NB], F32, tag="hf")

        nc.sync.dma_start(seqt[:], seqt_e[:])
        nc.sync.dma_start(wih0t[:], wih0t_e[:])
        nc.sync.dma_start(wih0b[:], wih0b_e[:])
        nc.sync.dma_start(wih1b[:], wih1b_e[:])
        nc.vector.memset(hbf[:], 0.0)
        nc.vector.memset(cst[:], 0.0)
        nc.vector.memset(ones16[:], 1.0)
        nc.scalar.activation(t1[:], cst[:], AF.Sigmoid)
        nc.scalar.activation(t2[:], cst[:], AF.Tanh)

        def gemm(lhsT_sb, bias_sb, rhs_of_k, rhs_ones, xp_dram, nk):
            """xp_dram[:, j*S + t] = sum_k lhsT(k,j).T @ rhs_k + bias_j (bf16)"""
            for ch in range(NGC):
                t0c = ch * GCH
                for j in range(NJ):
                    ps = pp.tile([128, GCH], F32, tag="gps", name="gps")
                    for k in range(nk):
                        nc.tensor.matmul(
                            ps[:],
                            lhsT_sb[:, (k * NJ + j) * 128 : (k * NJ + j + 1) * 128],
                            rhs_of_k(k, t0c, GCH),
                            start=(k == 0), stop=False,
                        )
                    nc.tensor.matmul(
                        ps[:],
                        bias_sb[0:1, 128 * j : 128 * (j + 1)],
                        rhs_ones[0:1, t0c : t0c + GCH],
                        start=False, stop=True,
                    )
                    nc.vector.tensor_copy(gstg[:], ps[:])  # cast to bf16
                    nc.sync.dma_start(
                        xp_dram[:, j * S + t0c : j * S + t0c + GCH], gstg[:]
                    )

        # --- GEMM0 ---
        seqt_r = seqt.rearrange("p (k t) -> p k t", k=KD)
        gemm(
            wih0t, wih0b,
            lambda k, t0c, chn: seqt_r[:, k, t0c : t0c + chn],
            ones16, xp0_d, KD,
        )
        gpool_cm.__exit__(None, None, None)
        wpool = ctx.enter_context(tc.tile_pool(name="wp", bufs=1))
        bigw = wpool.tile([128, NK * NJ * 128], BF16, tag="bigw")  # 144KB/part

        def cell_and_state(xs_ap, ps):
            nc.vector.tensor_add(gates[:], ps[:], xs_ap)
            nc.scalar.activation(sig[:], gates[:, 0:36], AF.Sigmoid)
            nc.scalar.activation(gt[:], gates[:, 36:48], AF.Tanh)
            nc.vector.tensor_mul(t1[:], sig[:, 0:36:3], gt[:])
            nc.vector.tensor_mul(t2[:], sig[:, 1:36:3], cst[:])
            nc.vector.tensor_add(cst[:], t1[:], t2[:])
            nc.scalar.activation(th[:], cst[:], AF.Tanh)
            nc.vector.tensor_mul(hf[:], sig[:, 2:36:3], th[:])
            return nc.vector.tensor_copy(hbf[:], hf[:])  # cast bf16

        def recurrence(whh_e, xp_dram, archive):
            # load this phase's recurrent weights into the big slot
            nc.sync.dma_start(bigw[:], whh_e[:])
            # prefetch first x_pre chunk
            xpc_r = [x.rearrange("p (j t) -> p j t", j=NJ) for x in xpc]
            xpd_r = xp_dram.rearrange("p (j t) -> p j t", j=NJ)
            nc.sync.dma_start(xpc[0][:], xpd_r[:, :, 0:CHK])
            for chu in range(NCHU):
                cur = xpc_r[chu % 2]
                if chu + 1 < NCHU:
                    nc.sync.dma_start(
                        xpc[(chu + 1) % 2][:],
                        xpd_r[:, :, (chu + 1) * CHK : (chu + 2) * CHK],
                    )
                with tc.For_i(0, CHK // unroll, 1) as it:
                    for u in range(unroll):
                        # tin = it*unroll + u (within chunk)
                        tin = it * unroll + u
                        ps = pp.tile([128, NJ], F32, tag="mv", name="mv")
                        for j in range(NJ):
                            for k in range(NK):
                                nc.tensor.matmul(
                                    ps[:, j : j + 1],
                                    bigw[:, (k * NJ + j) * 128 : (k * NJ + j + 1) * 128],
                                    hbf[:, k : k + 1],
                                    start=(k == 0), stop=(k == NK - 1),
                                )
                        xs = cur[:, :, bass.ds(tin, 1)].rearrange("p j one -> p (j one)")
                        cast = cell_and_state(xs, ps)
                        if archive:
                            nc.sync.dma_start(
                                arch[:, bass.ds((chu * CHK) * NB + (tin * NB), NB)],
                                hbf[:],
                            )

        recurrence(whh0t_e, xp0_d, archive=True)

        sv0 = nc.vector.tensor_copy(hfin[:, 0:NB], hf[:])
        sv1 = nc.vector.tensor_copy(hfin[:, NB : 2 * NB], cst[:])
        rst = nc.vector.memset(cst[:], 0.0)
        add_dep_helper(rst.ins, sv1.ins, reason="after save")
        rsh = nc.vector.memset(hbf[:], 0.0)

        # --- GEMM1: x_pre1 = H0 @ W_ih1.T + b1 ---
        nc.sync.dma_start(bigw[:], wih1t_e[:])
        arch_r = arch.rearrange("p (t k) -> p t k", k=NB)
        gemm(
            bigw, wih1b,
            lambda k, t0c, chn: arch_r[:, t0c : t0c + chn, k],
            ones16, xp1_d, NK,
        )

        recurrence(whh1t_e, xp1_d, archive=False)

        nc.vector.tensor_copy(hfin[:, 2 * NB : 3 * NB], hf[:])
        nc.vector.tensor_copy(hfin[:, 3 * NB : 4 * NB], cst[:])
        nc.sync.dma_start(hc_e[:], hfin[:])

    return nc


def assemble(results):
    h = np.zeros((2, H), np.float32)
    c = np.zeros((2, H), np.float32)
    hc = np.asarray(results[0]["hc"], dtype=np.float32)
    for blk in range(NB):
        u = 128 * blk
        h[0, u : u + 128] = hc[:, blk]
        c[0, u : u + 128] = hc[:, NB + blk]
        h[1, u : u + 128] = hc[:, 2 * NB + blk]
        c[1, u : u + 128] = hc[:, 3 * NB + blk]
    return h, c


def kernel(**inputs):
    """Full-input entry: build + compile + run on TRN2, return (h, c)."""
    from concourse.bass_utils import run_bass_kernel_spmd

    m, S = prep_inputs(**inputs)
    nc = build(S=S)
    nc.finalize()
    res = run_bass_kernel_spmd(nc, [m], [0])
    h, c = assemble(res.results)
    return h, c



# revision 2
# speedup vs baseline: 151.7930x; 151.7930x over previous
"""2-layer LSTM encoder (batch collapsed into recurrence) on TRN2 — v2.

Structure (single core; collectives are impractical per-step here):
  GEMM0: x_pre0 = seq @ W_ih0.T + b0   -> staged j-major in DRAM (bf16)
  rec0 : 4 segments of S/4 steps; x_pre0 segment streamed into SBUF;
         per step 576 LDW+MM pairs (48 gate-cols x 12 K-tiles, N=1) +
         9-op cell; h0 archived to SBUF (no HBM round trip).
  Per segment s: GEMM1(seg) computes x_pre1 = H0 @ W_ih1.T + b1 directly
         into SBUF (no DRAM staging), then rec1 over the segment.
  Weights are stationary in one 144KB/partition SBUF slot, reloaded per
  phase/segment (W_ih1 <-> W_hh1 swap per segment).

v1 emitted ~60k static instructions (24 chunk-unrolled loops); most of
the measured time was per-call host lowering proportional to program
size, not device time.  v2 is ~6k instructions with identical structure
at any S (so the small-S calibration run in test.py subtracts the host
overhead exactly), and avoids all per-step DRAM traffic.
"""

import sys

sys.path.insert(0, "/opt/trn_rl_repo")
import numpy as np
import ml_dtypes
import concourse.bass as bass
import concourse.bacc as bacc
import concourse.mybir as mybir
from concourse import tile
from concourse.tile_rust import add_dep_helper
from contextlib import ExitStack

F32 = mybir.dt.float32
BF16 = mybir.dt.bfloat16
AF = mybir.ActivationFunctionType

B, T, D, H = 16, 64, 256, 1536
NB = H // 128           # 12 unit blocks
NJ = 4 * NB             # 48 psum cols
NK = H // 128           # 12 K-tiles (H contraction)
KD = D // 128           # 2 K-tiles (D contraction)
SEG = 4                 # recurrence segments (x_pre SBUF-resident per seg)

_IOFF, _FOFF, _GOFF, _OOFF = 0, H, 2 * H, 3 * H


def col_gate(j):
    if j < 36:
        return [_IOFF, _FOFF, _OOFF][j % 3], j // 3
    return _GOFF, j - 36


def gate_rows(j):
    goff, blk = col_gate(j)
    return np.arange(goff + 128 * blk, goff + 128 * blk + 128)


def pack_lhsT(W, nk):
    out = np.zeros((128, nk * NJ * 128), dtype=W.dtype)
    for k in range(nk):
        for j in range(NJ):
            out[:, (k * NJ + j) * 128 : (k * NJ + j + 1) * 128] = W[
                gate_rows(j), 128 * k : 128 * (k + 1)
            ].T
    return out


def pack_biasT(b):
    out = np.zeros((128, NJ), dtype=np.float32)
    for j in range(NJ):
        out[:, j] = b[gate_rows(j)]
    return out


def prep_inputs(batch, W_ih0, W_hh0, b_ih0, b_hh0, W_ih1, W_hh1, b_ih1, b_hh1,
                S=None):
    bf = ml_dtypes.bfloat16
    seq = np.ascontiguousarray(
        np.asarray(batch)[:, 1:, :].transpose(1, 0, 2).reshape(-1, D)
    ).astype(np.float32)
    if S is not None:
        seq = seq[:S]
    S = seq.shape[0]
    seqt = np.ascontiguousarray(seq.T)
    b0 = (np.asarray(b_ih0) + np.asarray(b_hh0)).astype(np.float32)
    b1 = (np.asarray(b_ih1) + np.asarray(b_hh1)).astype(np.float32)
    m = {
        "seqt": np.ascontiguousarray(
            seqt.reshape(KD, 128, S).transpose(1, 0, 2).reshape(128, KD * S)
        ).astype(bf),
        "wih0t": pack_lhsT(np.asarray(W_ih0).astype(bf), KD),
        "whh0t": pack_lhsT(np.asarray(W_hh0).astype(bf), NK),
        "wih1t": pack_lhsT(np.asarray(W_ih1).astype(bf), NK),
        "whh1t": pack_lhsT(np.asarray(W_hh1).astype(bf), NK),
        "b0t": pack_biasT(b0),
        "b1t": pack_biasT(b1),
    }
    return m, S


def build(S=1008, **_ignored):
    assert S % SEG == 0
    LS = S // SEG

    nc = bacc.Bacc(
        "TRN2",
        target_bir_lowering=False,
        debug=False,
        detect_race_conditions=False,
        num_devices=1,
    )

    seqt_e = nc.declare_dram_parameter("seqt", [128, KD * S], BF16, isOutput=False)
    wih0t_e = nc.declare_dram_parameter("wih0t", [128, KD * NJ * 128], BF16, isOutput=False)
    whh0t_e = nc.declare_dram_parameter("whh0t", [128, NK * NJ * 128], BF16, isOutput=False)
    wih1t_e = nc.declare_dram_parameter("wih1t", [128, NK * NJ * 128], BF16, isOutput=False)
    whh1t_e = nc.declare_dram_parameter("whh1t", [128, NK * NJ * 128], BF16, isOutput=False)
    b0t_e = nc.declare_dram_parameter("b0t", [128, NJ], F32, isOutput=False)
    b1t_e = nc.declare_dram_parameter("b1t", [128, NJ], F32, isOutput=False)
    hc_e = nc.declare_dram_parameter("hc", [128, 4 * NB], F32, isOutput=True)

    # x_pre0 staging in DRAM, j-major: col j*S + t (bf16)
    xp0_d = nc.dram_tensor("xp0d", [128, NJ * S], BF16)

    with tile.TileContext(nc) as tc, ExitStack() as ctx:
        pool = ctx.enter_context(tc.tile_pool(name="main", bufs=1))
        gsp = ctx.enter_context(tc.tile_pool(name="gst", bufs=2))
        pp = ctx.enter_context(tc.tile_pool(name="ps", bufs=2, space="PSUM"))
        gp = ctx.enter_context(tc.tile_pool(name="gps", bufs=2, space="PSUM"))

        bigw = pool.tile([128, NK * NJ * 128], BF16, tag="bigw")   # 144KB
        seqt = pool.tile([128, KD * S], BF16, tag="seqt")
        arch = pool.tile([128, S * NB], BF16, tag="arch")          # H0 archive
        b0t = pool.tile([128, NJ], F32, tag="b0t")
        b1t = pool.tile([128, NJ], F32, tag="b1t")
        cst = pool.tile([128, NB], F32, tag="cst")
        hfin = pool.tile([128, 4 * NB], F32, tag="hfin")
        hbf = pool.tile([128, NB], BF16, tag="hbf")
        gates = pool.tile([128, NJ], F32, tag="gates")
        sig = pool.tile([128, 36], F32, tag="sig")
        gt = pool.tile([128, NB], F32, tag="gt")
        th = pool.tile([128, NB], F32, tag="th")
        t1 = pool.tile([128, NB], F32, tag="t1")
        t2 = pool.tile([128, NB], F32, tag="t2")
        hf = pool.tile([128, NB], F32, tag="hf")

        nc.sync.dma_start(seqt[:], seqt_e[:])
        nc.sync.dma_start(b0t[:], b0t_e[:])
        nc.sync.dma_start(b1t[:], b1t_e[:])
        nc.vector.memset(hbf[:], 0.0)
        nc.vector.memset(cst[:], 0.0)
        # warm the activation table (sigmoid_and_others holds both fns) so
        # the in-loop activations need no table load
        nc.scalar.activation(t1[:], cst[:], AF.Sigmoid)
        nc.scalar.activation(t2[:], cst[:], AF.Tanh)

        seqt_r = seqt.rearrange("p (k t) -> p k t", k=KD)
        arch_r = arch.rearrange("p (t k) -> p t k", k=NB)
        xpd_r = xp0_d.rearrange("p (j t) -> p j t", j=NJ)

        # --- GEMM0: x_pre0 = seq @ W_ih0.T + b0 -> DRAM (j-major) ---
        nc.sync.dma_start(bigw[:, 0 : KD * NJ * 128], wih0t_e[:])
        for sg in range(SEG):
            t0 = sg * LS
            for j in range(NJ):
                gps = gp.tile([128, LS], F32, tag="gps", name="gps")
                for k in range(KD):
                    nc.tensor.matmul(
                        gps[:],
                        bigw[:, (k * NJ + j) * 128 : (k * NJ + j + 1) * 128],
                        seqt_r[:, k, t0 : t0 + LS],
                        start=(k == 0), stop=(k == KD - 1),
                    )
                gstg = gsp.tile([128, LS], BF16, tag="gstg", name="gstg")
                nc.vector.tensor_scalar_add(gstg[:], gps[:], b0t[:, j : j + 1])
                nc.sync.dma_start(
                    xp0_d[:, j * S + t0 : j * S + t0 + LS], gstg[:]
                )

        xps = pool.tile([128, NJ * LS], BF16, tag="xslot")
        xps_r = xps.rearrange("p (j t) -> p j t", j=NJ)

        def cell(xs_ap):
            nc.vector.tensor_add(gates[:], xs_ap, gates_ps[:])
            nc.scalar.activation(sig[:], gates[:, 0:36], AF.Sigmoid)
            nc.scalar.activation(gt[:], gates[:, 36:48], AF.Tanh)
            nc.vector.tensor_mul(t1[:], sig[:, 0:36:3], gt[:])
            nc.vector.tensor_mul(t2[:], sig[:, 1:36:3], cst[:])
            nc.vector.tensor_add(cst[:], t1[:], t2[:])
            nc.scalar.activation(th[:], cst[:], AF.Tanh)
            nc.vector.tensor_mul(hf[:], sig[:, 2:36:3], th[:])
            return nc.vector.tensor_copy(hbf[:], hf[:])

        # --- rec0: 4 segments, x_pre0 streamed to SBUF, h0 -> arch ---
        nc.sync.dma_start(bigw[:], whh0t_e[:])
        for sg in range(SEG):
            t0 = sg * LS
            nc.sync.dma_start(xps[:], xpd_r[:, :, t0 : t0 + LS])
            with tc.For_i(0, LS, hint_engines=(mybir.EngineType.PE,),
                          name=f"recA{sg}") as tt:
                gates_ps = pp.tile([128, NJ], F32, tag="mv", name="mv")
                for j in range(NJ):
                    for k in range(NK):
                        nc.tensor.matmul(
                            gates_ps[:, j : j + 1],
                            bigw[:, (k * NJ + j) * 128 : (k * NJ + j + 1) * 128],
                            hbf[:, k : k + 1],
                            start=(k == 0), stop=(k == NK - 1),
                        )
                xs = xps_r[:, :, bass.ds(tt, 1)].rearrange("p j one -> p (j one)")
                cell(xs)
                nc.gpsimd.tensor_copy(
                    arch[:, bass.ds(t0 * NB + tt * NB, NB)], hbf[:]
                )

        sv0 = nc.vector.tensor_copy(hfin[:, 0:NB], hf[:])
        sv1 = nc.vector.tensor_copy(hfin[:, NB : 2 * NB], cst[:])
        rst = nc.vector.memset(cst[:], 0.0)
        add_dep_helper(rst.ins, sv1.ins, reason="after save")
        rsh = nc.vector.memset(hbf[:], 0.0)

        # --- layer 1: per segment GEMM1 (SBUF-resident x_pre1) + rec1 ---
        for sg in range(SEG):
            t0 = sg * LS
            nc.sync.dma_start(bigw[:], wih1t_e[:])
            for j in range(NJ):
                gps = gp.tile([128, LS], F32, tag="gps", name="gps")
                for k in range(NK):
                    nc.tensor.matmul(
                        gps[:],
                        bigw[:, (k * NJ + j) * 128 : (k * NJ + j + 1) * 128],
                        arch_r[:, t0 : t0 + LS, k],
                        start=(k == 0), stop=(k == NK - 1),
                    )
                nc.vector.tensor_scalar_add(
                    xps[:, j * LS : (j + 1) * LS], gps[:], b1t[:, j : j + 1]
                )
            nc.sync.dma_start(bigw[:], whh1t_e[:])
            with tc.For_i(0, LS, hint_engines=(mybir.EngineType.PE,),
                          name=f"recB{sg}") as tt:
                gates_ps = pp.tile([128, NJ], F32, tag="mv", name="mv")
                for j in range(NJ):
                    for k in range(NK):
                        nc.tensor.matmul(
                            gates_ps[:, j : j + 1],
                            bigw[:, (k * NJ + j) * 128 : (k * NJ + j + 1) * 128],
                            hbf[:, k : k + 1],
                            start=(k == 0), stop=(k == NK - 1),
                        )
                xs = xps_r[:, :, bass.ds(tt, 1)].rearrange("p j one -> p (j one)")
                cell(xs)

        nc.vector.tensor_copy(hfin[:, 2 * NB : 3 * NB], hf[:])
        nc.vector.tensor_copy(hfin[:, 3 * NB : 4 * NB], cst[:])
        nc.sync.dma_start(hc_e[:], hfin[:])

    return nc


def assemble(results):
    h = np.zeros((2, H), np.float32)
    c = np.zeros((2, H), np.float32)
    hc = np.asarray(results[0]["hc"], dtype=np.float32)
    for blk in range(NB):
        u = 128 * blk
        h[0, u : u + 128] = hc[:, blk]
        c[0, u : u + 128] = hc[:, NB + blk]
        h[1, u : u + 128] = hc[:, 2 * NB + blk]
        c[1, u : u + 128] = hc[:, 3 * NB + blk]
    return h, c


def kernel(**inputs):
    """Full-input entry: build + compile + run on TRN2, return (h, c)."""
    from concourse.bass_utils import run_bass_kernel_spmd

    m, S = prep_inputs(**inputs)
    nc = build(S=S)
    nc.finalize()
    res = run_bass_kernel_spmd(nc, [m], [0])
    h, c = assemble(res.results)
    return h, c


if __name__ == "__main__":
    pass
